# revision 23
# baseline (speedup 1.0000x reference)
"""Fused pre-LN transformer encoder layer (DeepSpeed-style) on 8 TRN2 NeuronCores.

Sharding strategy: token-parallel. The B*S = 4096 token rows are split 512
per core. Each core computes LN1 + its own Q/K/V slices; K^T and V slices
are all-gathered within the 4-core group that owns each batch (replica
groups [[0-3],[4-7]]); attention, the output projection, LN2 and the MLP
are then fully local to the core's 512 rows. Weights are replicated (each
core streams all of them once). Matmuls run in fp32r (full PE rate,
~1.5e-4 rounding); everything else is fp32.

Attention layout: scores are computed transposed (scoresT[k, q]) so the
exp'd scores feed the ctx matmul directly as the moving operand with V as
the stationary operand (no transposes of the S^2 probs). Softmax sums come
free from an extra ones-column appended to V; ctx rows are scaled by the
reciprocal at eviction time.
"""
import numpy as np
from contextlib import ExitStack

import concourse.bass as bass
import concourse.bacc as bacc
import concourse.mybir as mybir
import concourse.tile as tile
from concourse.bass_utils import run_bass_kernel_spmd
from concourse.masks import make_identity

F32 = mybir.dt.float32
F32R = mybir.dt.float32r
AF = mybir.ActivationFunctionType
OP = mybir.AluOpType

B, S, H, NH = 2, 2048, 1024, 16
DFF, HD = 4 * H, H // NH              # 4096, 64
EPS = 1e-12
NCORES = 8
T = B * S // NCORES                   # 512 tokens per core
NT = T // 128                         # 4 token tiles
NJ = H // 128                         # 8 hidden tiles
NF = DFF // 128                       # 32 ff tiles
NCH = 4                               # chunks (cores) per batch group
KT_N = H * T                          # elems in K^T slice
V_N = T * NH * (HD + 1)               # elems in V_aug slice
SCALE = float(1.0 / np.sqrt(HD))


def _col_ap(handle, n, offset=0):
    """DRAM vector -> [128, n] column-major AP (elem i*128+p -> [p, i])."""
    a = handle[:]
    return bass.AP(tensor=a.tensor, offset=offset, ap=[[1, 128], [128, n]])


def _bc_ap(handle, n, offset=0, parts=128):
    """DRAM vector [n] -> broadcast AP [parts, n]."""
    a = handle[:]
    return bass.AP(tensor=a.tensor, offset=offset, ap=[[0, parts], [1, n]])


def build_nc(sim_mode=False):
    """sim_mode=True: replace the AllGather with local DMA copies so the
    single-core TimelineSim can model the kernel."""
    nc = bacc.Bacc(None, num_devices=1 if sim_mode else NCORES)
    x = nc.dram_tensor("x_own", [T, H], F32, kind="ExternalInput")
    maskv = nc.dram_tensor("mask_own", [S], F32, kind="ExternalInput")
    qkvw = nc.dram_tensor("qkvw", [H, 3 * H], F32, kind="ExternalInput")
    qkvb = nc.dram_tensor("qkvb", [3 * H], F32, kind="ExternalInput")
    oww = nc.dram_tensor("ow", [H, H], F32, kind="ExternalInput")
    obv = nc.dram_tensor("ob", [H], F32, kind="ExternalInput")
    inter_w = nc.dram_tensor("inter_w", [H, DFF], F32, kind="ExternalInput")
    inter_b = nc.dram_tensor("inter_b", [DFF], F32, kind="ExternalInput")
    out_w = nc.dram_tensor("out_w", [DFF, H], F32, kind="ExternalInput")
    out_b = nc.dram_tensor("out_b", [H], F32, kind="ExternalInput")
    norm_w = nc.dram_tensor("norm_w", [H], F32, kind="ExternalInput")
    norm_b = nc.dram_tensor("norm_b", [H], F32, kind="ExternalInput")
    attn_nw = nc.dram_tensor("attn_nw", [H], F32, kind="ExternalInput")
    attn_nb = nc.dram_tensor("attn_nb", [H], F32, kind="ExternalInput")
    yout = nc.dram_tensor("y_own", [T, H], F32, kind="ExternalOutput")

    with tile.TileContext(nc) as tc, ExitStack() as top:
        const = top.enter_context(tc.tile_pool(name="const", bufs=1))
        dram = top.enter_context(tc.tile_pool(name="dram", bufs=1, space="DRAM"))

        ident = const.tile([128, 128], F32)
        make_identity(nc, ident)
        eps_t = const.tile([128, 1], F32)
        nc.vector.memset(eps_t, EPS)
        ones64 = const.tile([128, 64], F32)
        nc.vector.memset(ones64, 1.0)
        ones1r = const.tile([1, 64], F32R)
        nc.vector.tensor_copy(ones1r, ones64[0:1, :])

        def bc_tile(handle, offset=0):
            t_ = const.tile([128, H], F32, name=f"bc_{handle.name}_{offset}")
            nc.gpsimd.dma_start(t_, _bc_ap(handle, H, offset=offset))
            return t_

        nw_bc = bc_tile(norm_w)
        nb_bc = bc_tile(norm_b)
        anw_bc = bc_tile(attn_nw)
        anb_bc = bc_tile(attn_nb)
        ob_bc = bc_tile(obv)
        ob2_bc = bc_tile(out_b)
        vb_bc = bc_tile(qkvb, offset=2 * H)

        qb_col = const.tile([128, NJ], F32)
        nc.gpsimd.dma_start(qb_col, _col_ap(qkvb, NJ))
        kb_col = const.tile([128, NJ], F32)
        nc.gpsimd.dma_start(kb_col, _col_ap(qkvb, NJ, offset=H))
        ib_col = const.tile([128, NF], F32)
        nc.gpsimd.dma_start(ib_col, _col_ap(inter_b, NF))
        mask_col = const.tile([128, S // 128], F32)
        nc.gpsimd.dma_start(mask_col, _col_ap(maskv, S // 128))

        xs = const.tile([128, NT, H], F32)
        xr = x[:].rearrange("(t p) h -> p t h", p=128)
        for t in range(NT):
            nc.sync.dma_start(xs[:, t, :], xr[:, t, :])

        ctxT = top.enter_context(tc.tile_pool(name="ctxTp", bufs=1)).tile(
            [128, NJ, T], F32R)
        attn = top.enter_context(tc.tile_pool(name="attnp", bufs=1)).tile(
            [128, NT, H], F32)

        def layernorm(pool, out_ap, in_ap, w_bc, b_bc):
            stats = pool.tile([128, 2, 6], F32, tag="ln_stats", name="ln_stats")
            for g in range(2):
                nc.vector.bn_stats(stats[:, g, :], in_ap[:, g * 512:(g + 1) * 512])
            mv = pool.tile([128, 2], F32, tag="ln_mv", name="ln_mv")
            nc.vector.bn_aggr(mv, stats)
            rstd = pool.tile([128, 1], F32, tag="ln_rstd", name="ln_rstd")
            nc.scalar.activation(rstd, mv[:, 1:2], AF.Sqrt, bias=eps_t)
            nc.vector.reciprocal(rstd, rstd)
            nc.vector.tensor_scalar(out_ap, in_ap, mv[:, 0:1], rstd,
                                    OP.subtract, OP.mult)
            nc.vector.tensor_tensor(out_ap, out_ap, w_bc, OP.mult)
            nc.vector.tensor_tensor(out_ap, out_ap, b_bc, OP.add)

        def transpose_into(pspool, dst, src):
            # src [128, NT, H] fp32 tokens-major -> dst [128, NJ, T] fp32r
            for t in range(NT):
                for j in range(NJ):
                    pt = pspool.tile([128, 128], F32, tag="tp", name="tp")
                    nc.tensor.transpose(pt, src[:, t, j * 128:(j + 1) * 128], ident)
                    nc.vector.tensor_copy(dst[:, j, t * 128:(t + 1) * 128], pt)

        # ---------------- Phase A+B: LN1, h^T, QKV, AllGather ----------------
        with ExitStack() as ph:
            qt_pool = ph.enter_context(tc.tile_pool(name="qt", bufs=1))
            QT = qt_pool.tile([128, NJ, T], F32R)
            hT = qt_pool.tile([128, NJ, T], F32R)

            with ExitStack() as pha:
                lnp = pha.enter_context(tc.tile_pool(name="lnp", bufs=3))
                psT = pha.enter_context(tc.tile_pool(name="psT", bufs=3, space="PSUM"))
                hs_pool = pha.enter_context(tc.tile_pool(name="hsp", bufs=1))
                hs = hs_pool.tile([128, NT, H], F32)
                for t in range(NT):
                    layernorm(lnp, hs[:, t, :], xs[:, t, :], nw_bc, nb_bc)
                transpose_into(psT, hT, hs)

            cc_out_k = dram.tile([NCH, KT_N], F32, name="cc_out_k")
            cc_out_v = dram.tile([NCH, V_N], F32, name="cc_out_v")
            with ExitStack() as phb:
                wq = phb.enter_context(tc.tile_pool(name="wq", bufs=3))
                wv = phb.enter_context(tc.tile_pool(name="wv", bufs=2))
                psB = phb.enter_context(tc.tile_pool(name="psB", bufs=2, space="PSUM"))
                kv_pool = phb.enter_context(tc.tile_pool(name="kvp", bufs=1))
                KT = kv_pool.tile([128, NJ, T], F32R)
                Vag = kv_pool.tile([128, NT, NH, HD + 1], F32R)

                def ag(cc_in_ap, cc_out_t):
                    if sim_mode:
                        # timing stub: the real gather runs on TOPSP SDMA
                        # queues; locally only the own-chunk copy cost applies
                        nc.sync.dma_start(cc_out_t[0, :], cc_in_ap)
                    else:
                        nc.gpsimd.collective_compute(
                            "AllGather", OP.bypass,
                            replica_groups=[[0, 1, 2, 3], [4, 5, 6, 7]],
                            ins=[cc_in_ap], outs=[cc_out_t[:]])

                def qk_proj(dest, bcol, qk):
                    for j in range(NJ):
                        w_sb = wq.tile([128, NJ, 128], F32R, tag="wqk", name="wqk")
                        nc.sync.dma_start(
                            w_sb,
                            qkvw[:, qk * H + j * 128: qk * H + (j + 1) * 128]
                            .rearrange("(c p) d -> p c d", p=128).bitcast(F32R))
                        pq = psB.tile([128, T], F32, tag="pqk", name="pqk")
                        for c in range(NJ):
                            nc.tensor.matmul(pq, w_sb[:, c, :], hT[:, c, :],
                                             start=(c == 0), stop=(c == NJ - 1))
                        nc.vector.tensor_scalar_add(dest[:, j, :], pq, bcol[:, j:j + 1])

                # K^T first so AG_K (which gates the attention scores)
                # is in flight at the earliest possible point
                qk_proj(KT, kb_col, 1)
                cc_in_k = dram.tile([KT_N], F32, name="cc_in_k")
                nc.gpsimd.dma_start(
                    cc_in_k[:].rearrange("(j p t) -> p j t", j=NJ, p=128, t=T),
                    KT[:].bitcast(F32))
                ag(cc_in_k[:], cc_out_k)

                # V next, gather immediately
                for dh in range(2):
                    vw_sb = wv.tile([128, NJ, 512], F32R, tag="wv", name="wv")
                    nc.sync.dma_start(
                        vw_sb,
                        qkvw[:, 2 * H + dh * 512: 2 * H + (dh + 1) * 512]
                        .rearrange("(c p) d -> p c d", p=128).bitcast(F32R))
                    for t in range(NT):
                        pv = psB.tile([128, 512], F32, tag="pv", name="pv")
                        for c in range(NJ):
                            nc.tensor.matmul(pv, hT[:, c, t * 128:(t + 1) * 128],
                                             vw_sb[:, c, :],
                                             start=(c == 0), stop=(c == NJ - 1))
                        nc.vector.tensor_tensor(
                            Vag[:, t, dh * 8:(dh + 1) * 8, 0:HD],
                            pv.rearrange("p (h d) -> p h d", d=HD),
                            vb_bc[:, dh * 512:(dh + 1) * 512]
                            .rearrange("p (h d) -> p h d", d=HD),
                            OP.add)
                nc.vector.tensor_copy(
                    Vag[:, :, :, HD:HD + 1],
                    ones64.rearrange("p (t h d) -> p t h d", t=NT, h=NH, d=1))
                cc_in_v = dram.tile([V_N], F32, name="cc_in_v")
                nc.gpsimd.dma_start(
                    cc_in_v[:].rearrange("(t p h d) -> p t h d",
                                         t=NT, p=128, h=NH, d=HD + 1),
                    Vag[:].bitcast(F32))
                ag(cc_in_v[:], cc_out_v)

                # Q^T last — overlaps the in-flight gathers
                qk_proj(QT, qb_col, 0)

            # ---------------- Phase C: attention ----------------
            with ExitStack() as phc:
                kvs = phc.enter_context(tc.tile_pool(name="kvs", bufs=3))
                exp_pool = phc.enter_context(tc.tile_pool(name="exp", bufs=6))
                psS = phc.enter_context(tc.tile_pool(name="psS", bufs=2, space="PSUM"))
                psX = phc.enter_context(tc.tile_pool(name="psX", bufs=1, space="PSUM"))
                rpool = phc.enter_context(tc.tile_pool(name="rpool", bufs=3))

                for hp in range(NH // 2):           # head pairs
                    kts, vts = [], []
                    for c in range(NCH):
                        kt_c = kvs.tile([128, NT, 128], F32R, tag=f"kt{c}",
                                        name=f"kt{c}")
                        # K^T rows of this head pair: j = hp, chunk c tokens
                        nc.sync.dma_start(
                            kt_c,
                            cc_out_k[c, hp * (128 * T): (hp + 1) * (128 * T)]
                            .rearrange("(p t) -> p t", p=128)
                            .rearrange("p (s q) -> p s q", s=NT).bitcast(F32R))
                        kts.append(kt_c)
                        v_c = kvs.tile([128, NT, 2, HD + 1], F32R, tag=f"v{c}",
                                       name=f"v{c}")
                        # V rows: flat (t p h d) -> head pair slice h in {2hp, 2hp+1}
                        vsrc = cc_out_v[c, :].rearrange(
                            "(t p h d) -> p t h d", t=NT, p=128, h=NH, d=HD + 1)
                        nc.sync.dma_start(
                            v_c, vsrc[:, :, 2 * hp:2 * hp + 2, :].bitcast(F32R))
                        vts.append(v_c)

                    # both heads of the pair interleaved: the two 64-row score
                    # matmuls sit in different PE row groups and co-execute
                    pctxs = [psX.tile([HD + 1, T], F32, tag=f"pctx{sub}",
                                      name=f"pctx{sub}") for sub in range(2)]
                    for c in range(NCH):
                        for s_ in range(NT):
                            ks = c * NT + s_
                            # both heads' score tiles into one 2-bank PSUM so a
                            # single exp covers them (k partitions are shared)
                            psc = psS.tile([128, 2, T], F32, tag="psc",
                                           name="psc")
                            for sub in range(2):
                                p0 = sub * 64
                                nc.tensor.matmul(
                                    psc[:, sub, :],
                                    kts[c][p0:p0 + HD, s_, :],
                                    QT[p0:p0 + HD, hp, :],
                                    start=True, stop=True)
                            ex = exp_pool.tile([128, 2, T], F32R, tag="ex",
                                               name="ex")
                            nc.scalar.activation(
                                ex, psc, AF.Exp, scale=SCALE,
                                bias=mask_col[:, ks:ks + 1])
                            for sub in range(2):
                                nc.tensor.matmul(
                                    pctxs[sub], vts[c][:, s_, sub, :],
                                    ex[:, sub, :],
                                    start=(ks == 0), stop=(ks == NCH * NT - 1))
                    # softmax normalization at eviction: broadcast the
                    # reciprocal row across 64 partitions via a rank-1 fp32
                    # matmul (ones[1,64].T @ recip[1,T]) — low latency, no DMA
                    for sub in range(2):
                        p0 = sub * 64
                        pctx = pctxs[sub]
                        recip = rpool.tile([1, T], F32, tag="recip", name="recip")
                        nc.vector.reciprocal(recip, pctx[HD:HD + 1, :])
                        rec_r = rpool.tile([1, T], F32R, tag="rec_r", name="rec_r")
                        nc.vector.tensor_copy(rec_r, recip)
                        pbc = psS.tile([64, T], F32, tag="pbc", name="pbc")
                        nc.tensor.matmul(pbc, ones1r, rec_r,
                                         start=True, stop=True)
                        rb = rpool.tile([64, T], F32, tag="rb", name="rb")
                        nc.vector.tensor_copy(rb, pbc)
                        nc.vector.tensor_tensor(
                            ctxT[p0:p0 + 64, hp, :], pctx[0:HD, :], rb, OP.mult)

        # ---------------- Phase D: output proj + LN2 ----------------
        with ExitStack() as phd:
            h2_pool = phd.enter_context(tc.tile_pool(name="h2p", bufs=1))
            h2T = h2_pool.tile([128, NJ, T], F32R)
            with ExitStack() as phd1:
                wo = phd1.enter_context(tc.tile_pool(name="wo", bufs=2))
                psD = phd1.enter_context(tc.tile_pool(name="psD", bufs=2, space="PSUM"))
                lnp2 = phd1.enter_context(tc.tile_pool(name="lnp2", bufs=3))
                h2s_pool = phd1.enter_context(tc.tile_pool(name="h2sp", bufs=1))
                h2s = h2s_pool.tile([128, NT, H], F32)

                ow_sbs = []
                for n in range(2):
                    ow_sb = wo.tile([128, NJ, 512], F32R, tag=f"wo{n}", name="wo")
                    nc.sync.dma_start(
                        ow_sb,
                        oww[:, n * 512:(n + 1) * 512]
                        .rearrange("(c p) d -> p c d", p=128).bitcast(F32R))
                    ow_sbs.append(ow_sb)
                psT2 = phd1.enter_context(tc.tile_pool(name="psT2", bufs=3,
                                                       space="PSUM"))
                # t-outer so LN2/transpose of tile t pipelines with ow of t+1
                for t in range(NT):
                    for n in range(2):
                        po = psD.tile([128, 512], F32, tag="po", name="po")
                        for j in range(NJ):
                            nc.tensor.matmul(po, ctxT[:, j, t * 128:(t + 1) * 128],
                                             ow_sbs[n][:, j, :],
                                             start=(j == 0), stop=(j == NJ - 1))
                        nsl = slice(n * 512, (n + 1) * 512)
                        nc.vector.tensor_tensor(attn[:, t, nsl], po, xs[:, t, nsl],
                                                OP.add)
                        nc.vector.tensor_tensor(attn[:, t, nsl], attn[:, t, nsl],
                                                ob_bc[:, nsl], OP.add)
                    layernorm(lnp2, h2s[:, t, :], attn[:, t, :], anw_bc, anb_bc)
                    for j in range(NJ):
                        pt = psT2.tile([128, 128], F32, tag="tp2", name="tp2")
                        nc.tensor.transpose(pt, h2s[:, t, j * 128:(j + 1) * 128],
                                            ident)
                        nc.vector.tensor_copy(h2T[:, j, t * 128:(t + 1) * 128], pt)

            # ---------------- Phase E: MLP ----------------
            with ExitStack() as phe:
                outs_pool = phe.enter_context(tc.tile_pool(name="outsp", bufs=1))
                outs = outs_pool.tile([128, NT, H], F32)
                it_pool = phe.enter_context(tc.tile_pool(name="itp", bufs=1))
                interT = it_pool.tile([128, NF // 2, T], F32R)
                wi = phe.enter_context(tc.tile_pool(name="wi", bufs=3))
                wo2 = phe.enter_context(tc.tile_pool(name="wo2", bufs=3))

                for half in range(2):
                    f0 = half * (NF // 2)
                    with tc.tile_pool(name=f"psI{half}", bufs=2,
                                      space="PSUM") as psI:
                        for fg in range(4):
                            iw_sb = wi.tile([128, NJ, 512], F32R, tag="wi",
                                            name="wi")
                            col0 = half * (DFF // 2) + fg * 512
                            nc.sync.dma_start(
                                iw_sb,
                                inter_w[:, col0:col0 + 512]
                                .rearrange("(c p) d -> p c d", p=128).bitcast(F32R))
                            for fi in range(4):
                                f = fg * 4 + fi
                                # t-halves (N=256, still full fp32r rate) so
                                # production starts after LN2 of t0/t1 only
                                for th in range(2):
                                    tsl = slice(th * 256, (th + 1) * 256)
                                    pi = psI.tile([128, 256], F32, tag="pi",
                                                  name="pi")
                                    for c in range(NJ):
                                        nc.tensor.matmul(
                                            pi,
                                            iw_sb[:, c, fi * 128:(fi + 1) * 128],
                                            h2T[:, c, tsl],
                                            start=(c == 0), stop=(c == NJ - 1))
                                    nc.scalar.activation(
                                        interT[:, f, tsl], pi, AF.Gelu,
                                        bias=ib_col[:, f0 + f:f0 + f + 1])
                    with tc.tile_pool(name=f"psO{half}", bufs=1,
                                      space="PSUM") as psO:
                        pouts = [psO.tile([128, 512], F32, tag=f"pf{i}",
                                          name=f"pf{i}") for i in range(8)]
                        for f in range(NF // 2):
                            ow2_sb = wo2.tile([128, H], F32R, tag="wo2",
                                              name="wo2")
                            nc.sync.dma_start(
                                ow2_sb,
                                out_w[half * (DFF // 2) + f * 128:
                                      half * (DFF // 2) + (f + 1) * 128, :]
                                .bitcast(F32R))
                            for n in range(2):
                                for t in range(NT):
                                    nc.tensor.matmul(
                                        pouts[n * NT + t],
                                        interT[:, f, t * 128:(t + 1) * 128],
                                        ow2_sb[:, n * 512:(n + 1) * 512],
                                        start=(f == 0), stop=(f == NF // 2 - 1))
                        for n in range(2):
                            for t in range(NT):
                                nsl = slice(n * 512, (n + 1) * 512)
                                if half == 0:
                                    nc.vector.tensor_tensor(
                                        outs[:, t, nsl], pouts[n * NT + t],
                                        attn[:, t, nsl], OP.add)
                                else:
                                    nc.vector.tensor_tensor(
                                        outs[:, t, nsl], outs[:, t, nsl],
                                        pouts[n * NT + t], OP.add)
                                    nc.vector.tensor_tensor(
                                        outs[:, t, nsl], outs[:, t, nsl],
                                        ob2_bc[:, nsl], OP.add)
                                    nc.sync.dma_start(
                                        yout[:].rearrange("(t p) h -> p t h",
                                                          p=128)[:, t, nsl],
                                        outs[:, t, nsl])

    nc.finalize()
    return nc


_NC_CACHE = None


def _get_nc():
    global _NC_CACHE
    if _NC_CACHE is None:
        _NC_CACHE = build_nc()
    return _NC_CACHE


def make_in_maps(inputs):
    x = np.ascontiguousarray(np.asarray(inputs["x"], dtype=np.float32)
                             .reshape(B * S, H))
    mask = np.asarray(inputs["mask"], dtype=np.float32).reshape(B, S)
    full = {k: np.ascontiguousarray(np.asarray(inputs[k], dtype=np.float32))
            for k in ("qkvw", "qkvb", "ow", "ob", "inter_w", "inter_b",
                      "out_w", "out_b", "norm_w", "norm_b", "attn_nw",
                      "attn_nb")}
    in_maps = []
    for c in range(NCORES):
        m = dict(full)
        m["x_own"] = x[c * T:(c + 1) * T]
        m["mask_own"] = np.ascontiguousarray(mask[c // NCH])
        in_maps.append(m)
    return in_maps


def kernel(**inputs) -> np.ndarray:
    nc = _get_nc()
    res = run_bass_kernel_spmd(nc, make_in_maps(inputs),
                               core_ids=list(range(NCORES)), trace=False)
    out = np.concatenate([res.results[c]["y_own"] for c in range(NCORES)],
                         axis=0)
    return out.reshape(B, S, H)
